# revision 7
# baseline (speedup 1.0000x reference)
"""Trainium2 Bass kernel for nn_Convolution_1176821039249.

Computes out = base_map * mean_k box_k(x) for k in {3,5,7,9,11,13,15} with
replicate padding, on 8 NeuronCores, row-sharded with a 7-row halo.

Algorithm (per core):
  The total 2D kernel K(di,dj) = sum_k 1/(7k^2) * 1[|di|<=k//2] 1[|dj|<=k//2]
  is decomposed over the horizontal "wing" basis
      T_0 = x(center),  T_m(j) = x(j-m) + x(j+m)   (m = 1..7)
  so that  out = sum_{b=0..7} P_b-vertical-band applied to T_b, where
      P_b(d) = sum_{k: k//2 >= max(b,|d|)} 1/(7k^2).
  Engine balance (v2 cost model): wings {1,3,5} and {2,4,6} are two 3-slice
  fan adds on DVE (fp16 2x mode, 0.52 ns/free-elem); wing 7 runs on Pool
  (1.98 ns/elem effective); the vertical bands are 8 PSUM-accumulated banded
  matmuls on the PE per 114-row tile (0.42 ns/out-col/stream); ACT drains
  PSUM to fp16; the base_map multiply is split Pool/DVE at column QM and
  software-pipelined two strips behind the drain so neither in-order engine
  queue stalls.  I/O is fp16 end to end (host pads/casts in, upcasts out).
"""

import numpy as np

F16 = np.float16

H = W = 4096
PAD = 7
N_CORES = 8
RPC = H // N_CORES          # 512 output rows per core
TILE_M = 114                # output rows per row tile (128 - 2*PAD)
N_TILES = 5                 # 4 * 114 + 56 = 512
LAST_M = RPC - 4 * TILE_M   # 56
STRIP = 2048                # output cols per strip
N_STRIPS = W // STRIP       # 2
CHUNK = 512                 # matmul N chunk (one PSUM bank of fp32)
QM = 1344                   # base-mul split: Pool does [0,QM), DVE [QM,STRIP)
FLUSH_LAG = 1               # strips between drain and mul+store
KERNEL_SIZES = (3, 5, 7, 9, 11, 13, 15)

_CACHE = {}


def _bands_np() -> np.ndarray:
    """lhsT band matrices, [128, 8*TILE_M] fp16.

    Band b column i row p holds P_b(p - i - 7): the vertical pyramid profile
    applied to wing tensor T_b.
    """
    w = {k: 1.0 / (7.0 * k * k) for k in KERNEL_SIZES}
    P = np.zeros((8, 15), dtype=np.float64)
    for b in range(8):
        for d in range(-7, 8):
            P[b, d + 7] = sum(w[k] for k in KERNEL_SIZES if k // 2 >= max(b, abs(d)))
    M = np.zeros((128, 8 * TILE_M), dtype=np.float64)
    for b in range(8):
        for i in range(TILE_M):
            p_lo = i  # d = p - i - 7 in [-7, 7]; P is indexed at d + 7 = p - i
            for p in range(p_lo, p_lo + 15):
                M[p, b * TILE_M + i] = P[b, p - i]
    return M.astype(F16)


def _build_nc():
    import concourse.bass as bass
    import concourse.mybir as mybir
    import concourse.tile as tile

    dt = mybir.dt
    SHARD_R = RPC + 2 * PAD     # 526
    SHARD_C = W + 2 * PAD       # 4110

    nc = bass.Bass()
    xb_d = nc.declare_dram_parameter("xb", [SHARD_R, SHARD_C], dt.float16, isOutput=False)
    base_d = nc.declare_dram_parameter("base", [RPC, W], dt.float16, isOutput=False)
    bands_d = nc.declare_dram_parameter("bands", [128, 8 * TILE_M], dt.float16, isOutput=False)
    out_d = nc.declare_dram_parameter("out", [RPC, W], dt.float16, isOutput=True)

    with tile.TileContext(nc) as tc:
        with (
            tc.tile_pool(name="const", bufs=1) as constp,
            tc.tile_pool(name="xin", bufs=2) as xpool,
            tc.tile_pool(name="wings", bufs=2) as apool,
            tc.tile_pool(name="io", bufs=2) as iopool,
            tc.tile_pool(name="psum", bufs=2, space="PSUM") as psump,
        ):
            bands_sb = constp.tile([128, 8 * TILE_M], dt.float16, name="bands_sb")

            def fan(src, rows, start, n, step, width=STRIP):
                # [rows, n, width] view: slice i starts at column start + i*step
                # (overlapping windows; innermost stays stride-1 so the fp16
                # 2x_1p DVE mode is preserved)
                v = src[:rows, start:start + width].unsqueeze(1)
                lst = v.ap
                lst[1] = (step, n)
                v.ap = lst
                return v

            pending = []

            def flush_one(tail=False):
                r0, c0, M, acc, bt = pending.pop(0)
                if not tail:
                    nc.gpsimd.tensor_mul(acc[:M, :QM], acc[:M, :QM],
                                         bt[:M, c0:c0 + QM])
                    nc.vector.tensor_mul(acc[:M, QM:], acc[:M, QM:],
                                         bt[:M, c0 + QM:c0 + STRIP])
                    nc.sync.dma_start(out_d[r0:r0 + M, c0:c0 + STRIP], acc[:M, :])
                else:
                    # tail: short splits so the kernel drain isn't serialized
                    # behind one long Pool multiply
                    nc.gpsimd.tensor_mul(acc[:M, :CHUNK], acc[:M, :CHUNK],
                                         bt[:M, c0:c0 + CHUNK])
                    nc.vector.tensor_mul(acc[:M, CHUNK:], acc[:M, CHUNK:],
                                         bt[:M, c0 + CHUNK:c0 + STRIP])
                    nc.sync.dma_start(out_d[r0:r0 + M, c0:c0 + CHUNK], acc[:M, :CHUNK])
                    nc.sync.dma_start(out_d[r0:r0 + M, c0 + CHUNK:c0 + STRIP],
                                      acc[:M, CHUNK:])

            for t in range(N_TILES):
                M = TILE_M if t < N_TILES - 1 else LAST_M
                K = M + 2 * PAD
                r0 = t * TILE_M
                # One dedicated slot per row tile: no slot reuse, so the HWDGE
                # load DMAs carry no sync waits (walrus 1-wait DMA limit).
                # Two half loads per tile so strip 0 compute starts early.
                xt = xpool.tile([128, SHARD_C], dt.float16, tag="xt", name="xt", bufs=N_TILES)
                HALF = STRIP + 2 * PAD
                nc.sync.dma_start(xt[:K, :HALF], xb_d[r0:r0 + K, :HALF])
                nc.sync.dma_start(xt[:K, HALF:], xb_d[r0:r0 + K, HALF:])
                if t == 0:
                    # constants load after the first x half: off the fill path
                    nc.sync.dma_start(bands_sb[:], bands_d[:])
                bt = iopool.tile([128, W], dt.float16, tag="bt", name="bt", bufs=N_TILES)
                nc.sync.dma_start(bt[:M, :], base_d[r0:r0 + M, :])

                for s in range(N_STRIPS):
                    c0 = s * STRIP
                    last = (t == N_TILES - 1 and s == N_STRIPS - 1)
                    # wing 7 on Pool first: the PE consumes it last (band 7)
                    a7 = apool.tile([128, STRIP], dt.float16, tag="a7", name="a7")
                    nc.gpsimd.tensor_add(a7[:K, :], xt[:K, c0:c0 + STRIP],
                                         xt[:K, c0 + 14:c0 + 14 + STRIP])
                    # wings {1,3,5} and {2,4,6}: 3-slice fan adds on DVE
                    a135 = apool.tile([128, 3, STRIP], dt.float16, tag="a135", name="a135")
                    nc.vector.tensor_add(a135[:K], fan(xt, K, c0 + 6, 3, -2),
                                         fan(xt, K, c0 + 8, 3, 2))
                    a246 = apool.tile([128, 3, STRIP], dt.float16, tag="a246", name="a246")
                    nc.vector.tensor_add(a246[:K], fan(xt, K, c0 + 5, 3, -2),
                                         fan(xt, K, c0 + 9, 3, 2))
                    wing = {1: a135[:, 0], 3: a135[:, 1], 5: a135[:, 2],
                            2: a246[:, 0], 4: a246[:, 1], 6: a246[:, 2]}

                    def rhs_of(b):
                        if b == 0:
                            return xt[:K, c0 + PAD:c0 + PAD + STRIP]
                        if b == 7:
                            return a7[:K, :]
                        return wing[b][:K, :]

                    ps = psump.tile([128, STRIP], dt.float32, tag="ps", name="ps")
                    acc = iopool.tile([128, STRIP], dt.float16, tag="acc",
                                      name="acc", bufs=FLUSH_LAG + 1)
                    if not last:
                        for b in (0, 1, 3, 2, 4, 6, 5, 7):
                            rhs = rhs_of(b)
                            lhsT = bands_sb[:K, b * TILE_M:b * TILE_M + M]
                            for c in range(STRIP // CHUNK):
                                nc.tensor.matmul(
                                    ps[:M, c * CHUNK:(c + 1) * CHUNK],
                                    lhsT,
                                    rhs[:, c * CHUNK:(c + 1) * CHUNK],
                                    start=(b == 0),
                                    stop=(b == 7),
                                )
                        # ACT drains PSUM to fp16 (Pool cannot read PSUM)
                        nc.scalar.copy(acc[:M, :], ps[:M, :])
                        pending.append((r0, c0, M, acc, bt))
                        if len(pending) > FLUSH_LAG:
                            flush_one()
                    else:
                        # tail strip: chunk-major so drain+mul+store pipeline
                        # per 512-col chunk instead of waiting for the strip
                        if pending:
                            flush_one()
                        for c in range(STRIP // CHUNK):
                            cc = slice(c * CHUNK, (c + 1) * CHUNK)
                            for b in (0, 1, 3, 2, 4, 6, 5, 7):
                                lhsT = bands_sb[:K, b * TILE_M:b * TILE_M + M]
                                nc.tensor.matmul(
                                    ps[:M, cc],
                                    lhsT,
                                    rhs_of(b)[:, cc],
                                    start=(b == 0),
                                    stop=(b == 7),
                                )
                            nc.scalar.copy(acc[:M, cc], ps[:M, cc])
                            nc.vector.tensor_mul(acc[:M, cc], acc[:M, cc],
                                                 bt[:M, c0 + c * CHUNK:c0 + (c + 1) * CHUNK])
                            nc.sync.dma_start(
                                out_d[r0:r0 + M, c0 + c * CHUNK:c0 + (c + 1) * CHUNK],
                                acc[:M, cc])
            while pending:
                flush_one()
    return nc


def _split_sync_waits(nc):
    """Walrus codegen only supports one sync wait per instruction; hoist
    extra waits onto injected NoOps on the instruction's engine (identical
    semantics: the sequencer blocks at the NoOp first, then at the
    instruction).  DMA instructions are issued from their engine's
    sequencer stream, so the same hoisting applies to them.
    """
    import concourse.mybir as mybir

    n_nops = 0
    for fn in nc.m.functions:
        for bb in fn.blocks:
            new = []
            for inst in bb.instructions:
                si = inst.sync_info
                if si is not None and si.on_wait and len(si.on_wait) > 1:
                    waits = list(si.on_wait)
                    hoist, keep = waits[:-1], waits[-1:]
                    for w in hoist:
                        nop = mybir.InstNoOp(name=f"{inst.name}-w{n_nops}", ins=[], outs=[])
                        nop.engine = inst.engine
                        nop.sync_info = mybir.SyncInfo(on_wait=[w], on_update=[])
                        new.append(nop)
                        n_nops += 1
                    if hoist:
                        inst.sync_info = mybir.SyncInfo(
                            on_wait=keep, on_update=list(si.on_update))
                new.append(inst)
            bb.instructions = new
    return n_nops


def _get_nc():
    if "nc" not in _CACHE:
        nc = _build_nc()
        _split_sync_waits(nc)
        _CACHE["nc"] = nc
    return _CACHE["nc"]


def _run(x: np.ndarray, base_map: np.ndarray, trace: bool = False):
    from concourse.bass_utils import run_bass_kernel_spmd

    nc = _get_nc()
    xp = np.pad(np.asarray(x, dtype=np.float32), PAD, mode="edge").astype(F16)
    base_map = np.ascontiguousarray(np.asarray(base_map, dtype=np.float32).astype(F16))
    bands = _bands_np()
    in_maps = []
    for c in range(N_CORES):
        r0 = c * RPC
        in_maps.append({
            "xb": np.ascontiguousarray(xp[r0:r0 + RPC + 2 * PAD]),
            "base": base_map[r0:r0 + RPC],
            "bands": bands,
        })
    res = run_bass_kernel_spmd(nc, in_maps, list(range(N_CORES)), trace=trace)
    out = np.concatenate([res.results[c]["out"] for c in range(N_CORES)], axis=0)
    return out[None, None].astype(np.float32), res


def kernel(x: np.ndarray, base_map: np.ndarray) -> np.ndarray:
    out, _ = _run(x, base_map, trace=False)
    return out


# revision 8
# speedup vs baseline: 1.0359x; 1.0359x over previous
"""Trainium2 Bass kernel for nn_Convolution_1176821039249.

Computes out = base_map * mean_k box_k(x) for k in {3,5,7,9,11,13,15} with
replicate padding, on 8 NeuronCores, row-sharded with a 7-row halo.

Algorithm (per core):
  The total 2D kernel K(di,dj) = sum_k 1/(7k^2) * 1[|di|<=k//2] 1[|dj|<=k//2]
  is decomposed over the horizontal "wing" basis
      T_0 = x(center),  T_m(j) = x(j-m) + x(j+m)   (m = 1..7)
  so that  out = sum_{b=0..7} P_b-vertical-band applied to T_b, where
      P_b(d) = sum_{k: k//2 >= max(b,|d|)} 1/(7k^2).
  Engine balance (v2 cost model): wings {1,3,5} and {2,4,6} are two 3-slice
  fan adds on DVE (fp16 2x mode, 0.52 ns/free-elem); wing 7 runs on Pool
  (1.98 ns/elem effective); the vertical bands are 8 PSUM-accumulated banded
  matmuls on the PE per 114-row tile (0.42 ns/out-col/stream); ACT drains
  PSUM to fp16; the base_map multiply is split Pool/DVE at column QM and
  software-pipelined two strips behind the drain so neither in-order engine
  queue stalls.  I/O is fp16 end to end (host pads/casts in, upcasts out).
"""

import numpy as np

F16 = np.float16

H = W = 4096
PAD = 7
N_CORES = 8
RPC = H // N_CORES          # 512 output rows per core
TILE_M = 114                # output rows per row tile (128 - 2*PAD)
N_TILES = 5                 # 4 * 114 + 56 = 512
LAST_M = RPC - 4 * TILE_M   # 56
STRIP = 2048                # output cols per strip
N_STRIPS = W // STRIP       # 2
CHUNK = 512                 # matmul N chunk (one PSUM bank of fp32)
QM = 1344                   # base-mul split: Pool does [0,QM), DVE [QM,STRIP)
FLUSH_LAG = 2               # strips between drain and mul+store
KERNEL_SIZES = (3, 5, 7, 9, 11, 13, 15)

_CACHE = {}


def _bands_np() -> np.ndarray:
    """lhsT band matrices, [128, 8*TILE_M] fp16.

    Band b column i row p holds P_b(p - i - 7): the vertical pyramid profile
    applied to wing tensor T_b.
    """
    w = {k: 1.0 / (7.0 * k * k) for k in KERNEL_SIZES}
    P = np.zeros((8, 15), dtype=np.float64)
    for b in range(8):
        for d in range(-7, 8):
            P[b, d + 7] = sum(w[k] for k in KERNEL_SIZES if k // 2 >= max(b, abs(d)))
    M = np.zeros((128, 8 * TILE_M), dtype=np.float64)
    for b in range(8):
        for i in range(TILE_M):
            p_lo = i  # d = p - i - 7 in [-7, 7]; P is indexed at d + 7 = p - i
            for p in range(p_lo, p_lo + 15):
                M[p, b * TILE_M + i] = P[b, p - i]
    return M.astype(F16)


def _build_nc():
    import concourse.bass as bass
    import concourse.mybir as mybir
    import concourse.tile as tile

    dt = mybir.dt
    SHARD_R = RPC + 2 * PAD     # 526
    SHARD_C = W + 2 * PAD       # 4110

    nc = bass.Bass()
    xb_d = nc.declare_dram_parameter("xb", [SHARD_R, SHARD_C], dt.float16, isOutput=False)
    base_d = nc.declare_dram_parameter("base", [RPC, W], dt.float16, isOutput=False)
    bands_d = nc.declare_dram_parameter("bands", [128, 8 * TILE_M], dt.float16, isOutput=False)
    out_d = nc.declare_dram_parameter("out", [RPC, W], dt.float16, isOutput=True)

    with tile.TileContext(nc) as tc:
        with (
            tc.tile_pool(name="const", bufs=1) as constp,
            tc.tile_pool(name="xin", bufs=2) as xpool,
            tc.tile_pool(name="wings", bufs=2) as apool,
            tc.tile_pool(name="io", bufs=2) as iopool,
            tc.tile_pool(name="psum", bufs=2, space="PSUM") as psump,
        ):
            bands_sb = constp.tile([128, 8 * TILE_M], dt.float16, name="bands_sb")
            # constants + base_map loads issue from the idle ACT queue so the
            # SP queue's first DMA is the x tile the whole pipeline waits on
            nc.scalar.dma_start(bands_sb[:], bands_d[:])

            def fan(src, rows, start, n, step, width=STRIP):
                # [rows, n, width] view: slice i starts at column start + i*step
                # (overlapping windows; innermost stays stride-1 so the fp16
                # 2x_1p DVE mode is preserved)
                v = src[:rows, start:start + width].unsqueeze(1)
                lst = v.ap
                lst[1] = (step, n)
                v.ap = lst
                return v

            pending = []

            def flush_one(tail=False):
                r0, c0, M, acc, bt = pending.pop(0)
                if not tail:
                    nc.gpsimd.tensor_mul(acc[:M, :QM], acc[:M, :QM],
                                         bt[:M, c0:c0 + QM])
                    nc.vector.tensor_mul(acc[:M, QM:], acc[:M, QM:],
                                         bt[:M, c0 + QM:c0 + STRIP])
                    nc.sync.dma_start(out_d[r0:r0 + M, c0:c0 + STRIP], acc[:M, :])
                else:
                    # tail: short splits so the kernel drain isn't serialized
                    # behind one long Pool multiply
                    nc.gpsimd.tensor_mul(acc[:M, :CHUNK], acc[:M, :CHUNK],
                                         bt[:M, c0:c0 + CHUNK])
                    nc.vector.tensor_mul(acc[:M, CHUNK:], acc[:M, CHUNK:],
                                         bt[:M, c0 + CHUNK:c0 + STRIP])
                    nc.sync.dma_start(out_d[r0:r0 + M, c0:c0 + CHUNK], acc[:M, :CHUNK])
                    nc.sync.dma_start(out_d[r0:r0 + M, c0 + CHUNK:c0 + STRIP],
                                      acc[:M, CHUNK:])

            for t in range(N_TILES):
                M = TILE_M if t < N_TILES - 1 else LAST_M
                K = M + 2 * PAD
                r0 = t * TILE_M
                # One dedicated slot per row tile: no slot reuse, so the HWDGE
                # load DMAs carry no sync waits (walrus 1-wait DMA limit).
                # Two half loads per tile so strip 0 compute starts early.
                xt = xpool.tile([128, SHARD_C], dt.float16, tag="xt", name="xt", bufs=N_TILES)
                HALF = STRIP + 2 * PAD
                nc.sync.dma_start(xt[:K, :HALF], xb_d[r0:r0 + K, :HALF])
                nc.sync.dma_start(xt[:K, HALF:], xb_d[r0:r0 + K, HALF:])
                bt = iopool.tile([128, W], dt.float16, tag="bt", name="bt", bufs=N_TILES)
                nc.scalar.dma_start(bt[:M, :], base_d[r0:r0 + M, :])

                for s in range(N_STRIPS):
                    c0 = s * STRIP
                    last = (t == N_TILES - 1 and s == N_STRIPS - 1)
                    # wing 7 on Pool first: the PE consumes it last (band 7)
                    a7 = apool.tile([128, STRIP], dt.float16, tag="a7", name="a7")
                    nc.gpsimd.tensor_add(a7[:K, :], xt[:K, c0:c0 + STRIP],
                                         xt[:K, c0 + 14:c0 + 14 + STRIP])
                    # wings {1,3,5} and {2,4,6}: 3-slice fan adds on DVE
                    a135 = apool.tile([128, 3, STRIP], dt.float16, tag="a135", name="a135")
                    nc.vector.tensor_add(a135[:K], fan(xt, K, c0 + 6, 3, -2),
                                         fan(xt, K, c0 + 8, 3, 2))
                    a246 = apool.tile([128, 3, STRIP], dt.float16, tag="a246", name="a246")
                    nc.vector.tensor_add(a246[:K], fan(xt, K, c0 + 5, 3, -2),
                                         fan(xt, K, c0 + 9, 3, 2))
                    wing = {1: a135[:, 0], 3: a135[:, 1], 5: a135[:, 2],
                            2: a246[:, 0], 4: a246[:, 1], 6: a246[:, 2]}

                    def rhs_of(b):
                        if b == 0:
                            return xt[:K, c0 + PAD:c0 + PAD + STRIP]
                        if b == 7:
                            return a7[:K, :]
                        return wing[b][:K, :]

                    ps = psump.tile([128, STRIP], dt.float32, tag="ps", name="ps")
                    acc = iopool.tile([128, STRIP], dt.float16, tag="acc",
                                      name="acc", bufs=FLUSH_LAG + 1)
                    if not last:
                        for b in (0, 1, 3, 2, 4, 6, 5, 7):
                            rhs = rhs_of(b)
                            lhsT = bands_sb[:K, b * TILE_M:b * TILE_M + M]
                            for c in range(STRIP // CHUNK):
                                nc.tensor.matmul(
                                    ps[:M, c * CHUNK:(c + 1) * CHUNK],
                                    lhsT,
                                    rhs[:, c * CHUNK:(c + 1) * CHUNK],
                                    start=(b == 0),
                                    stop=(b == 7),
                                )
                        # ACT drains PSUM to fp16 (Pool cannot read PSUM)
                        nc.scalar.copy(acc[:M, :], ps[:M, :])
                        pending.append((r0, c0, M, acc, bt))
                        if len(pending) > FLUSH_LAG:
                            flush_one()
                    else:
                        # tail strip: the two pending muls hide under this
                        # strip's PE work; then chunked drain+mul+store so the
                        # kernel drain pipelines at 512-col granularity
                        while pending:
                            flush_one()
                        for b in (0, 1, 3, 2, 4, 6, 5, 7):
                            rhs = rhs_of(b)
                            lhsT = bands_sb[:K, b * TILE_M:b * TILE_M + M]
                            for c in range(STRIP // CHUNK):
                                nc.tensor.matmul(
                                    ps[:M, c * CHUNK:(c + 1) * CHUNK],
                                    lhsT,
                                    rhs[:, c * CHUNK:(c + 1) * CHUNK],
                                    start=(b == 0),
                                    stop=(b == 7),
                                )
                        for c in range(STRIP // CHUNK):
                            cc = slice(c * CHUNK, (c + 1) * CHUNK)
                            nc.scalar.copy(acc[:M, cc], ps[:M, cc])
                            nc.vector.tensor_mul(acc[:M, cc], acc[:M, cc],
                                                 bt[:M, c0 + c * CHUNK:c0 + (c + 1) * CHUNK])
                            nc.sync.dma_start(
                                out_d[r0:r0 + M, c0 + c * CHUNK:c0 + (c + 1) * CHUNK],
                                acc[:M, cc])
            while pending:
                flush_one()
    return nc


def _split_sync_waits(nc):
    """Walrus codegen only supports one sync wait per instruction; hoist
    extra waits onto injected NoOps on the instruction's engine (identical
    semantics: the sequencer blocks at the NoOp first, then at the
    instruction).  DMA instructions are issued from their engine's
    sequencer stream, so the same hoisting applies to them.
    """
    import concourse.mybir as mybir

    n_nops = 0
    for fn in nc.m.functions:
        for bb in fn.blocks:
            new = []
            for inst in bb.instructions:
                si = inst.sync_info
                if si is not None and si.on_wait and len(si.on_wait) > 1:
                    waits = list(si.on_wait)
                    hoist, keep = waits[:-1], waits[-1:]
                    for w in hoist:
                        nop = mybir.InstNoOp(name=f"{inst.name}-w{n_nops}", ins=[], outs=[])
                        nop.engine = inst.engine
                        nop.sync_info = mybir.SyncInfo(on_wait=[w], on_update=[])
                        new.append(nop)
                        n_nops += 1
                    if hoist:
                        inst.sync_info = mybir.SyncInfo(
                            on_wait=keep, on_update=list(si.on_update))
                new.append(inst)
            bb.instructions = new
    return n_nops


def _get_nc():
    if "nc" not in _CACHE:
        nc = _build_nc()
        _split_sync_waits(nc)
        _CACHE["nc"] = nc
    return _CACHE["nc"]


def _run(x: np.ndarray, base_map: np.ndarray, trace: bool = False):
    from concourse.bass_utils import run_bass_kernel_spmd

    nc = _get_nc()
    xp = np.pad(np.asarray(x, dtype=np.float32), PAD, mode="edge").astype(F16)
    base_map = np.ascontiguousarray(np.asarray(base_map, dtype=np.float32).astype(F16))
    bands = _bands_np()
    in_maps = []
    for c in range(N_CORES):
        r0 = c * RPC
        in_maps.append({
            "xb": np.ascontiguousarray(xp[r0:r0 + RPC + 2 * PAD]),
            "base": base_map[r0:r0 + RPC],
            "bands": bands,
        })
    res = run_bass_kernel_spmd(nc, in_maps, list(range(N_CORES)), trace=trace)
    out = np.concatenate([res.results[c]["out"] for c in range(N_CORES)], axis=0)
    return out[None, None].astype(np.float32), res


def kernel(x: np.ndarray, base_map: np.ndarray) -> np.ndarray:
    out, _ = _run(x, base_map, trace=False)
    return out
